# revision 1
# baseline (speedup 1.0000x reference)
# Causal self-attention (B=4, T=2048, C=1024, H=16, Dh=64) on 8 NeuronCores.
#
# Sharding: core (b, g) = batch b, head-group g (8 heads). Tensor-parallel over
# heads + data-parallel over batch. Each core computes a full [T, C] partial
# projection output; host sums the two head-group partials per batch.
#
# Per-core device program (all matmul operands bf16, fp32 PSUM accumulate):
#   1. QK proj (transposed): QK_T[o, t] for o in [Q(512) | K(512)], streamed
#      over 4 query-chunks of 512 tokens.
#   2. V proj (untransposed): V[t, h*65 + d] with a ones column per head at
#      d=64 (gives softmax denominator for free in the PV matmul).
#   3. Attention per head-pair: S_T[k, q] = K_T.T @ Q_T blocks of
#      [128 keys x 512 queries]; 2 heads packed in the PE array via
#      base-partition 0/64 (row tiling). exp on ScalarE (PSUM->SBUF, bf16),
#      block-causal masks applied by DVE multiply on diagonal blocks,
#      y_T[d, q] (+ denom row 64) = V_aug.T @ P_T accumulated over key blocks.
#   4. Normalize fused into PSUM evac: z = y_T * (1/denom broadcast), bf16.
#   5. Proj: out[t, :] = sum_c z[c, t] * Wp[c, :], fp32 out, DMA to HBM.

import numpy as np
import ml_dtypes

B, T, C = 4, 2048, 1024
H, DH = 16, 64
HL = 8            # heads per core
DL = HL * DH      # 512 local head dims
NCORES = 8
TCH = 512         # token chunk (query chunk)
NQC = T // TCH    # 4
NKT = T // 128    # 16 key tiles

BF16 = ml_dtypes.bfloat16

_CACHE = {}


def _build_nc():
    import concourse.bass as bass
    import concourse.tile as tile
    import concourse.mybir as mybir
    from concourse import bacc

    f32 = mybir.dt.float32
    bf16 = mybir.dt.bfloat16
    EXP = mybir.ActivationFunctionType.Exp

    nc = bacc.Bacc("TRN2", target_bir_lowering=False, debug=False)

    # ---- I/O ----
    xt_d = nc.dram_tensor("xt", [C, T], bf16, kind="ExternalInput")
    wq_d = nc.dram_tensor("wq", [C, 1024], bf16, kind="ExternalInput")  # [c, Q|K]
    wv_d = nc.dram_tensor("wv", [C, DL], bf16, kind="ExternalInput")
    wp_d = nc.dram_tensor("wp", [DL, C], bf16, kind="ExternalInput")
    out_d = nc.dram_tensor("out", [T, C], f32, kind="ExternalOutput")

    # causal block masks: mask[i, m, j] = 1 if j >= i + 128*m else 0
    mi = np.arange(128)[:, None, None]
    mm = np.arange(4)[None, :, None]
    mj = np.arange(TCH)[None, None, :]
    mask_np = (mj >= mi + 128 * mm).astype(BF16)
    mask_d = nc.inline_tensor(np.ascontiguousarray(mask_np), name="cmask")

    with tile.TileContext(nc) as tc:
        with (
            tc.tile_pool(name="persist", bufs=1) as persist,
            tc.tile_pool(name="xtp", bufs=4) as xtp,
            tc.tile_pool(name="pp", bufs=8) as pp,
            tc.tile_pool(name="ytp", bufs=6) as ytp,
            tc.tile_pool(name="recp", bufs=6) as recp,
            tc.tile_pool(name="rbp", bufs=6) as rbp,
            tc.tile_pool(name="outp", bufs=4) as outp,
            tc.tile_pool(name="ps", bufs=3, space="PSUM") as psp,
            tc.tile_pool(name="pvps", bufs=3, space="PSUM") as pvpsp,
        ):
            # ---- persistent tiles (wq first: first QK matmul depends on it) ----
            wq_sb = persist.tile([128, 8, 1024], bf16, tag="wq", name="wq")
            wv_sb = persist.tile([128, 8, DL], bf16, tag="wv", name="wv")
            wp_sb = persist.tile([128, 4, 1024], bf16, tag="wp", name="wp")
            mask_sb = persist.tile([128, 4, TCH], bf16, tag="mask", name="mask")
            # few big 3D-AP DMAs; cc0 of wq separate so the first matmul
            # unblocks as early as possible
            wq_r = wq_d.rearrange("(cc p) o -> p cc o", p=128)
            wv_r = wv_d.rearrange("(cc p) o -> p cc o", p=128)
            wp_r = wp_d.rearrange("(cc p) o -> p cc o", p=128)


            # QK_T pair tiles: [128, 2*TCH]; halves = o-tiles 2jp, 2jp+1
            # (jp<2: Q head-pairs; jp>=2: K head-pairs)
            qk_sb = [[persist.tile([128, 2 * TCH], bf16, tag=f"qk_{j}_{q}", name=f"qk_{j}_{q}")
                      for q in range(NQC)] for j in range(4)]
            # V pair tiles: [128 keys, 2 key-tiles, 8 heads, 65]
            v_sb = [persist.tile([128, 2, HL, DH + 1], bf16, tag=f"v_{t}", name=f"v_{t}")
                    for t in range(NKT // 2)]
            # z = normalized y_T: per (head-pair, qc): rows 0-63 head 2p, 64-127 head 2p+1
            z_sb = [[persist.tile([128, TCH], bf16, tag=f"z_{p}_{q}", name=f"z_{p}_{q}")
                     for q in range(NQC)] for p in range(4)]

            # input DMAs ordered by first use: the transfer engine is a
            # serial resource, so earliest-needed bytes go first
            xt_r = xt_d.rearrange("(cc p) t -> p cc t", p=128)
            xt_tiles = [xtp.tile([128, 8, TCH], bf16, tag="xt", name="xt")
                        for _ in range(NQC)]
            nc.sync.dma_start(out=wq_sb[:, 0, :], in_=wq_r[:, 0, :])
            nc.sync.dma_start(out=xt_tiles[0][:, 0, :], in_=xt_r[:, 0, 0:TCH])
            for cc in range(1, 8):
                nc.sync.dma_start(out=wq_sb[:, cc, :], in_=wq_r[:, cc, :])
                nc.sync.dma_start(out=xt_tiles[0][:, cc, :], in_=xt_r[:, cc, 0:TCH])
            nc.sync.dma_start(out=wv_sb, in_=wv_r)
            nc.sync.dma_start(out=mask_sb, in_=mask_d[:])
            nc.sync.dma_start(out=wp_sb, in_=wp_r)
            for qc in range(1, NQC):
                nc.sync.dma_start(out=xt_tiles[qc],
                                  in_=xt_r[:, :, qc * TCH:(qc + 1) * TCH])

            # Filler units are half-sized (one 512-wide PSUM bank, 8 or 4
            # matmuls, ~0.9-1.7us) so they interleave finely against the
            # constant ~185ns/key-tile exp-overhead deficit of the S chain.
            def emit_proj_half(qc, u, half, tg="psh"):
                tt = qc * 4 + u
                ps_t = psp.tile([128, 512], f32, tag=tg, name="ps",
                                bufs=(1 if tg == "psh" else 2))
                for cp in range(4):
                    nc.tensor.matmul(
                        ps_t,
                        lhsT=z_sb[cp][qc][:, u * 128:(u + 1) * 128],
                        rhs=wp_sb[:, cp, half * 512:(half + 1) * 512],
                        start=(cp == 0), stop=(cp == 3))
                o_t = outp.tile([128, 512], f32, tag="o", name="o")
                nc.vector.tensor_copy(out=o_t, in_=ps_t)
                nc.sync.dma_start(
                    out=out_d[tt * 128:(tt + 1) * 128, half * 512:(half + 1) * 512],
                    in_=o_t)

            def emit_proj(qc):
                for u in range(4):
                    for half in range(2):
                        emit_proj_half(qc, u, half, tg="ps")

            def emit_qk_half(qc, jp, half, tg="psh"):
                xt_t = xt_tiles[qc]
                j = 2 * jp + half
                ps_t = psp.tile([128, 512], f32, tag=tg, name="ps",
                                bufs=(1 if tg == "psh" else 2))
                for cc in range(8):
                    nc.tensor.matmul(
                        ps_t,
                        lhsT=wq_sb[:, cc, j * 128:(j + 1) * 128],
                        rhs=xt_t[:, cc, :],
                        start=(cc == 0), stop=(cc == 7))
                nc.vector.tensor_copy(
                    out=qk_sb[jp][qc][:, half * 512:(half + 1) * 512], in_=ps_t)

            def emit_v_half(qc, up, half, tg="psh"):
                xt_t = xt_tiles[qc]
                ps_t = psp.tile([128, 512], f32, tag=tg, name="ps",
                                bufs=(1 if tg == "psh" else 2))
                for cc in range(8):
                    nc.tensor.matmul(
                        ps_t,
                        lhsT=xt_t[:, cc, (2 * up + half) * 128:(2 * up + half + 1) * 128],
                        rhs=wv_sb[:, cc, :],
                        start=(cc == 0), stop=(cc == 7))
                tp = qc * 2 + up
                nc.vector.tensor_copy(out=v_sb[tp][:, half, :, 0:DH], in_=ps_t)
                nc.gpsimd.memset(v_sb[tp][:, half, :, DH:DH + 1], 1.0)

            def emit_qkv(qc):
                for jp in range(4):
                    for half in range(2):
                        emit_qk_half(qc, jp, half, tg="ps")
                for up in range(2):
                    for half in range(2):
                        emit_v_half(qc, up, half, tg="ps")

            emit_qkv(0)

            for qc in range(NQC):
                # ---------- attention for query chunk qc ----------
                # Fillers interleaved between head-pairs keep PE busy while
                # attention waits on the exp pipeline: proj of the previous
                # chunk + QKV of the next chunk.
                # Filler budget balanced against each window's exp time:
                # att(3) has no next-chunk QKV, so both proj(1) and proj(2)
                # are deferred into it; att(2) runs on QKV(3) alone.
                fillers = []
                if qc == 3:
                    fillers += [lambda pq=pq, u=u, h=h: emit_proj_half(pq, u, h)
                                for pq in (0, 1, 2) for u in range(4) for h in range(2)]
                if qc + 1 < NQC:
                    fillers += [lambda jp=jp, h=h: emit_qk_half(qc + 1, jp, h)
                                for jp in range(4) for h in range(2)]
                    fillers += [lambda up=up, h=h: emit_v_half(qc + 1, up, h)
                                for up in range(2) for h in range(2)]

                nkt = 4 * qc + 4  # causal: key tiles 0..4qc+3
                total_iters = 4 * nkt
                it = emitted = 0
                for hp in range(4):
                    y_ps = [pvpsp.tile([DH + 1, TCH], f32, tag="pv", name="pv") for _ in range(2)]
                    for kt in range(nkt):
                        # s_t halves = the two heads of the pair at the SAME
                        # key tile, so both share causal offset m and the exp
                        # can be narrowed with one 3D AP on diagonal blocks
                        m = kt - 4 * qc
                        w0 = 128 * m if m > 0 else 0
                        s_t = psp.tile([128, 1024], f32, tag="ps", name="ps", bufs=2)
                        for hh in range(2):
                            nc.tensor.matmul(
                                s_t[:, hh * 512 + w0:(hh + 1) * 512],
                                lhsT=qk_sb[2 + hp // 2][kt // 4][
                                    hh * 64:(hh + 1) * 64,
                                    (hp % 2) * 512 + (kt % 4) * 128:
                                    (hp % 2) * 512 + ((kt % 4) + 1) * 128],
                                rhs=qk_sb[hp // 2][qc][hh * 64:(hh + 1) * 64,
                                                       (hp % 2) * 512 + w0:(hp % 2 + 1) * 512],
                                start=True, stop=True)
                        p_t = pp.tile([128, 1024], bf16, tag="p", name="p")
                        if w0:
                            nc.scalar.activation(
                                out=p_t.rearrange("p (h w) -> p h w", h=2)[:, :, w0:],
                                in_=s_t.rearrange("p (h w) -> p h w", h=2)[:, :, w0:],
                                func=EXP)
                        else:
                            nc.scalar.activation(out=p_t, in_=s_t, func=EXP)
                        if m >= 0:  # diagonal block: apply causal mask
                            for hh in range(2):
                                nc.vector.tensor_mul(
                                    p_t[:, hh * 512 + w0:(hh + 1) * 512],
                                    p_t[:, hh * 512 + w0:(hh + 1) * 512],
                                    mask_sb[:, m, w0:])
                        # spread filler work (prev proj + next QKV) evenly
                        # through the attention window, emitted BETWEEN the S
                        # and PV matmuls so it can hide the exp latency in the
                        # FIFO engine stream
                        it += 1
                        while emitted < len(fillers) * it // total_iters:
                            fillers[emitted]()
                            emitted += 1
                        for hh in range(2):
                            h = 2 * hp + hh
                            nc.tensor.matmul(
                                y_ps[hh][:, w0:],
                                lhsT=v_sb[kt // 2][:, kt % 2, h, :],
                                rhs=p_t[:, hh * 512 + w0:(hh + 1) * 512],
                                start=(kt == 0), stop=(kt == nkt - 1))
                    # evacuate PSUM early (frees the pv slot), then normalize:
                    # 1/denom broadcast to 64 partitions on idle GpSimd
                    for hh in range(2):
                        yt_t = ytp.tile([DH + 1, TCH], mybir.dt.float32, tag="yt", name="yt")
                        rec_t = recp.tile([1, TCH], mybir.dt.float32, tag="rec", name="rec")
                        nc.vector.reciprocal(out=rec_t, in_=y_ps[hh][DH:DH + 1, :])
                        nc.vector.tensor_copy(out=yt_t[0:DH, :], in_=y_ps[hh][0:DH, :])
                        rb_t = rbp.tile([64, TCH], mybir.dt.float32, tag="rb", name="rb")
                        nc.gpsimd.partition_broadcast(rb_t, rec_t)
                        nc.vector.tensor_mul(
                            z_sb[hp][qc][hh * 64:(hh + 1) * 64, :],
                            yt_t[0:DH, :],
                            rb_t)
                while emitted < len(fillers):
                    fillers[emitted]()
                    emitted += 1

            emit_proj(NQC - 1)

    nc.compile()
    return nc


def _get_nc():
    if "nc" not in _CACHE:
        _CACHE["nc"] = _build_nc()
    return _CACHE["nc"]


def _prep_core_inputs(x, W_qkv, W_proj, b, g):
    xt = np.ascontiguousarray(x[b].T).astype(BF16)                    # [C, T]
    wq_rows = W_qkv[g * DL:(g + 1) * DL, :] * (1.0 / np.sqrt(DH))     # Q, pre-scaled
    wk_rows = W_qkv[C + g * DL:C + (g + 1) * DL, :]
    wq = np.ascontiguousarray(np.concatenate([wq_rows, wk_rows], 0).T).astype(BF16)
    wv = np.ascontiguousarray(W_qkv[2 * C + g * DL:2 * C + (g + 1) * DL, :].T).astype(BF16)
    wp = np.ascontiguousarray(W_proj[:, g * DL:(g + 1) * DL].T).astype(BF16)
    return {"xt": xt, "wq": wq, "wv": wv, "wp": wp}


def kernel(x, W_qkv, W_proj, _trace=False):
    from concourse.bass_utils import run_bass_kernel_spmd

    x = np.asarray(x, dtype=np.float32)
    W_qkv = np.asarray(W_qkv, dtype=np.float32)
    W_proj = np.asarray(W_proj, dtype=np.float32)

    nc = _get_nc()
    in_maps = [_prep_core_inputs(x, W_qkv, W_proj, cid // 2, cid % 2)
               for cid in range(NCORES)]
    res = run_bass_kernel_spmd(nc, in_maps, core_ids=list(range(NCORES)),
                               trace=_trace)
    _CACHE["last_results"] = res
    out = np.empty((B, T, C), dtype=np.float32)
    for b in range(B):
        out[b] = res.results[2 * b]["out"] + res.results[2 * b + 1]["out"]
    return out



# revision 4
# speedup vs baseline: 1.0730x; 1.0730x over previous
# Causal self-attention (B=4, T=2048, C=1024, H=16, Dh=64) on 8 NeuronCores.
#
# Sharding: core (b, g) = batch b, head-group g (8 heads). Tensor-parallel over
# heads + data-parallel over batch. Each core computes a full [T, C] partial
# projection output; host sums the two head-group partials per batch.
#
# Per-core device program (all matmul operands bf16, fp32 PSUM accumulate):
#   1. QK proj (transposed): QK_T[o, t] for o in [Q(512) | K(512)], streamed
#      over 4 query-chunks of 512 tokens.
#   2. V proj (untransposed): V[t, h*65 + d] with a ones column per head at
#      d=64 (gives softmax denominator for free in the PV matmul).
#   3. Attention per head-pair: S_T[k, q] = K_T.T @ Q_T blocks of
#      [128 keys x 512 queries]; 2 heads packed in the PE array via
#      base-partition 0/64 (row tiling). exp on ScalarE (PSUM->SBUF, bf16),
#      tri-mask applied by DVE multiply on the 128-col diagonal window only.
#   4. PV flipped: y[q, d] (+ denom col 64) = P_T.T @ V_aug per q-subtile of
#      128 (full 128 output partitions, causal subtile skipping), accumulated
#      over key tiles into bank-padded PSUM slots. One PSUM accumulation
#      group per 2KB bank: start on the bank's first write, stop on its last.
#   5. Normalize with per-partition scalar 1/denom on DVE (bf16 z[q, c-pair]),
#      then DMA-XBAR transpose each [128, 128] block back to z_T[c, q].
#   6. Proj: out[t, :] = sum_c z_T[c, t] * Wp[c, :], fp32 out, DMA to HBM.

import numpy as np
import ml_dtypes

B, T, C = 4, 2048, 1024
H, DH = 16, 64
HL = 8            # heads per core
DL = HL * DH      # 512 local head dims
NCORES = 8
TCH = 512         # token chunk (query chunk)
NQC = T // TCH    # 4
NKT = T // 128    # 16 key tiles

BF16 = ml_dtypes.bfloat16

_CACHE = {}


def _build_nc():
    import concourse.bass as bass
    import concourse.tile as tile
    import concourse.mybir as mybir
    from concourse import bacc

    f32 = mybir.dt.float32
    bf16 = mybir.dt.bfloat16
    EXP = mybir.ActivationFunctionType.Exp

    nc = bacc.Bacc("TRN2", target_bir_lowering=False, debug=False)

    # ---- I/O ----
    xt_d = nc.dram_tensor("xt", [C, T], bf16, kind="ExternalInput")
    wq_d = nc.dram_tensor("wq", [C, 1024], bf16, kind="ExternalInput")  # [c, Q|K]
    wv_d = nc.dram_tensor("wv", [C, DL], bf16, kind="ExternalInput")
    wp_d = nc.dram_tensor("wp", [DL, C], bf16, kind="ExternalInput")
    out_d = nc.dram_tensor("out", [T, C], f32, kind="ExternalOutput")

    # lower-triangular block mask: tri[i, j] = 1 if j >= i (applied on the
    # single 128-col diagonal window of each diagonal S block)
    mi = np.arange(128)[:, None]
    mj = np.arange(128)[None, :]
    tri_np = (mj >= mi).astype(BF16)
    tri_d = nc.inline_tensor(np.ascontiguousarray(tri_np), name="tri")

    with tile.TileContext(nc) as tc:
        with (
            tc.tile_pool(name="persist", bufs=1) as persist,
            tc.tile_pool(name="xtp", bufs=4) as xtp,
            tc.tile_pool(name="pp", bufs=8) as pp,
            tc.tile_pool(name="zqp", bufs=6) as zqp,
            tc.tile_pool(name="recp", bufs=3) as recp,
            tc.tile_pool(name="outp", bufs=4) as outp,
            tc.tile_pool(name="sps", bufs=2, space="PSUM") as spsp,
            tc.tile_pool(name="fps", bufs=2, space="PSUM") as fpsp,
            tc.tile_pool(name="pvps", bufs=1, space="PSUM") as pvpsp,
        ):
            # ---- persistent tiles (wq first: first QK matmul depends on it) ----
            wq_sb = persist.tile([128, 8, 1024], bf16, tag="wq", name="wq")
            wv_sb = persist.tile([128, 8, DL], bf16, tag="wv", name="wv")
            wp_sb = persist.tile([128, 4, 1024], bf16, tag="wp", name="wp")
            tri_sb = persist.tile([128, 128], bf16, tag="tri", name="tri")
            # few big 3D-AP DMAs; cc0 of wq separate so the first matmul
            # unblocks as early as possible
            wq_r = wq_d.rearrange("(cc p) o -> p cc o", p=128)
            wv_r = wv_d.rearrange("(cc p) o -> p cc o", p=128)
            wp_r = wp_d.rearrange("(cc p) o -> p cc o", p=128)

            # QK_T pair tiles: [128, 2*TCH]; halves = o-tiles 2jp, 2jp+1
            # (jp<2: Q head-pairs; jp>=2: K head-pairs)
            qk_sb = [[persist.tile([128, 2 * TCH], bf16, tag=f"qk_{j}_{q}", name=f"qk_{j}_{q}")
                      for q in range(NQC)] for j in range(4)]
            # V pair tiles: [128 keys, 2 key-tiles, 8 heads, 65]
            v_sb = [persist.tile([128, 2, HL, DH + 1], bf16, tag=f"v_{t}", name=f"v_{t}")
                    for t in range(NKT // 2)]
            # z_T = normalized y_T: per (head-pair, qc): rows 0-63 head 2p,
            # 64-127 head 2p+1, [128 c, TCH q]
            z_sb = [[persist.tile([128, TCH], bf16, tag=f"z_{p}_{q}", name=f"z_{p}_{q}")
                     for q in range(NQC)] for p in range(4)]

            # input DMAs ordered by first use: the transfer engine is a
            # serial resource, so earliest-needed bytes go first
            xt_r = xt_d.rearrange("(cc p) t -> p cc t", p=128)
            xt_tiles = [xtp.tile([128, 8, TCH], bf16, tag="xt", name="xt")
                        for _ in range(NQC)]
            nc.sync.dma_start(out=wq_sb[:, 0, :], in_=wq_r[:, 0, :])
            nc.sync.dma_start(out=xt_tiles[0][:, 0, :], in_=xt_r[:, 0, 0:TCH])
            for cc in range(1, 8):
                nc.sync.dma_start(out=wq_sb[:, cc, :], in_=wq_r[:, cc, :])
                nc.sync.dma_start(out=xt_tiles[0][:, cc, :], in_=xt_r[:, cc, 0:TCH])
            nc.sync.dma_start(out=wv_sb, in_=wv_r)
            nc.sync.dma_start(out=tri_sb, in_=tri_d[:])
            nc.sync.dma_start(out=wp_sb, in_=wp_r)
            for qc in range(1, NQC):
                nc.sync.dma_start(out=xt_tiles[qc],
                                  in_=xt_r[:, :, qc * TCH:(qc + 1) * TCH])

            # Filler units are half-sized (one 512-wide PSUM bank, 8 or 4
            # matmuls) so they interleave finely against the exp-latency
            # deficit of the S chain.
            def emit_proj_half(qc, u, half):
                tt = qc * 4 + u
                ps_t = fpsp.tile([128, 512], f32, tag="f", name="ps")
                for cp in range(4):
                    nc.tensor.matmul(
                        ps_t,
                        lhsT=z_sb[cp][qc][:, u * 128:(u + 1) * 128],
                        rhs=wp_sb[:, cp, half * 512:(half + 1) * 512],
                        start=(cp == 0), stop=(cp == 3))
                o_t = outp.tile([128, 512], f32, tag="o", name="o")
                nc.vector.tensor_copy(out=o_t, in_=ps_t)
                nc.sync.dma_start(
                    out=out_d[tt * 128:(tt + 1) * 128, half * 512:(half + 1) * 512],
                    in_=o_t)

            def emit_proj(qc):
                for u in range(4):
                    for half in range(2):
                        emit_proj_half(qc, u, half)

            def emit_qk_half(qc, jp, half):
                xt_t = xt_tiles[qc]
                j = 2 * jp + half
                ps_t = fpsp.tile([128, 512], f32, tag="f", name="ps")
                for cc in range(8):
                    nc.tensor.matmul(
                        ps_t,
                        lhsT=wq_sb[:, cc, j * 128:(j + 1) * 128],
                        rhs=xt_t[:, cc, :],
                        start=(cc == 0), stop=(cc == 7))
                nc.vector.tensor_copy(
                    out=qk_sb[jp][qc][:, half * 512:(half + 1) * 512], in_=ps_t)

            def emit_v_half(qc, up, half):
                xt_t = xt_tiles[qc]
                ps_t = fpsp.tile([128, 512], f32, tag="f", name="ps")
                for cc in range(8):
                    nc.tensor.matmul(
                        ps_t,
                        lhsT=xt_t[:, cc, (2 * up + half) * 128:(2 * up + half + 1) * 128],
                        rhs=wv_sb[:, cc, :],
                        start=(cc == 0), stop=(cc == 7))
                tp = qc * 2 + up
                nc.vector.tensor_copy(out=v_sb[tp][:, half, :, 0:DH], in_=ps_t)
                nc.gpsimd.memset(v_sb[tp][:, half, :, DH:DH + 1], 1.0)

            def emit_qkv(qc):
                for jp in range(4):
                    for half in range(2):
                        emit_qk_half(qc, jp, half)
                for up in range(2):
                    for half in range(2):
                        emit_v_half(qc, up, half)

            emit_qkv(0)

            for qc in range(NQC):
                # ---------- attention for query chunk qc ----------
                # Fillers interleaved between head-pairs keep PE busy while
                # attention waits on the exp pipeline.
                fillers = []
                if qc > 0:
                    fillers += [lambda pq=qc - 1, u=u, h=h: emit_proj_half(pq, u, h)
                                for u in range(4) for h in range(2)]
                if qc + 1 < NQC:
                    fillers += [lambda jp=jp, h=h: emit_qk_half(qc + 1, jp, h)
                                for jp in range(4) for h in range(2)]
                    fillers += [lambda up=up, h=h: emit_v_half(qc + 1, up, h)
                                for up in range(2) for h in range(2)]

                nkt = 4 * qc + 4  # causal: key tiles 0..4qc+3
                total_iters = 4 * nkt
                it = emitted = 0
                for hp in range(4):
                    # PV accumulators: [128 q, hh, s, 65 of 128] f32; the s
                    # slots of one hh share a 2KB bank = one accumulation
                    # group (start on first write, stop on last)
                    y_ps = pvpsp.tile([128, 2, 4, 128], f32, tag="pv", name="pv")
                    for kt in range(nkt):
                        # s_t halves = the two heads of the pair at the SAME
                        # key tile, so both share causal offset m and the exp
                        # can be narrowed with one 3D AP on diagonal blocks
                        m = kt - 4 * qc
                        w0 = 128 * m if m > 0 else 0
                        s_t = spsp.tile([128, 1024], f32, tag="s", name="s")
                        for hh in range(2):
                            nc.tensor.matmul(
                                s_t[:, hh * 512 + w0:(hh + 1) * 512],
                                lhsT=qk_sb[2 + hp // 2][kt // 4][
                                    hh * 64:(hh + 1) * 64,
                                    (hp % 2) * 512 + (kt % 4) * 128:
                                    (hp % 2) * 512 + ((kt % 4) + 1) * 128],
                                rhs=qk_sb[hp // 2][qc][hh * 64:(hh + 1) * 64,
                                                       (hp % 2) * 512 + w0:(hp % 2 + 1) * 512],
                                start=True, stop=True)
                        p_t = pp.tile([128, 1024], bf16, tag="p", name="p")
                        if w0:
                            nc.scalar.activation(
                                out=p_t.rearrange("p (h w) -> p h w", h=2)[:, :, w0:],
                                in_=s_t.rearrange("p (h w) -> p h w", h=2)[:, :, w0:],
                                func=EXP)
                        else:
                            nc.scalar.activation(out=p_t, in_=s_t, func=EXP)
                        if m >= 0:  # diagonal block: mask the 128-col window
                            for hh in range(2):
                                nc.vector.tensor_mul(
                                    p_t[:, hh * 512 + w0:hh * 512 + w0 + 128],
                                    p_t[:, hh * 512 + w0:hh * 512 + w0 + 128],
                                    tri_sb)
                        # spread filler work (prev proj + next QKV) evenly
                        # through the attention window, emitted BETWEEN the S
                        # and PV matmuls so it can hide the exp latency in the
                        # FIFO engine stream
                        it += 1
                        while emitted < len(fillers) * it // total_iters:
                            fillers[emitted]()
                            emitted += 1
                        # flipped PV: per q-subtile s of 128, skip subtiles
                        # that precede this key tile (causal)
                        for hh in range(2):
                            h = 2 * hp + hh
                            for s in range(4):
                                if 4 * qc + s < kt:
                                    continue
                                nc.tensor.matmul(
                                    y_ps[:, hh, s, 0:DH + 1],
                                    lhsT=p_t[:, hh * 512 + s * 128:hh * 512 + (s + 1) * 128],
                                    rhs=v_sb[kt // 2][:, kt % 2, h, :],
                                    start=(kt == 0 and s == 0),
                                    stop=(kt == nkt - 1 and s == 3))
                    # normalize: z[q, hh*64+d] = y[q, hh, s, d] / y[q, hh, s, 64]
                    rec_t = recp.tile([128, 2, 4], f32, tag="rec", name="rec")
                    for hh in range(2):
                        nc.vector.reciprocal(out=rec_t[:, hh, :],
                                             in_=y_ps[:, hh, :, DH])
                    for s in range(4):
                        zq_t = zqp.tile([128, 2, DH], bf16, tag="zq", name="zq")
                        nc.vector.tensor_mul(
                            zq_t,
                            y_ps[:, :, s, 0:DH],
                            rec_t[:, :, s:s + 1].broadcast_to([128, 2, DH]))
                        # z_T[c, q] block via DMA-XBAR transpose (no PE/PSUM)
                        nc.sync.dma_start_transpose(
                            out=z_sb[hp][qc][:, s * 128:(s + 1) * 128],
                            in_=zq_t.rearrange("p a b -> p (a b)"))
                while emitted < len(fillers):
                    fillers[emitted]()
                    emitted += 1

            emit_proj(NQC - 1)

    nc.compile()
    return nc


def _get_nc():
    if "nc" not in _CACHE:
        _CACHE["nc"] = _build_nc()
    return _CACHE["nc"]


def _prep_core_inputs(x, W_qkv, W_proj, b, g):
    xt = np.ascontiguousarray(x[b].T).astype(BF16)                    # [C, T]
    wq_rows = W_qkv[g * DL:(g + 1) * DL, :] * (1.0 / np.sqrt(DH))     # Q, pre-scaled
    wk_rows = W_qkv[C + g * DL:C + (g + 1) * DL, :]
    wq = np.ascontiguousarray(np.concatenate([wq_rows, wk_rows], 0).T).astype(BF16)
    wv = np.ascontiguousarray(W_qkv[2 * C + g * DL:2 * C + (g + 1) * DL, :].T).astype(BF16)
    wp = np.ascontiguousarray(W_proj[:, g * DL:(g + 1) * DL].T).astype(BF16)
    return {"xt": xt, "wq": wq, "wv": wv, "wp": wp}


def kernel(x, W_qkv, W_proj, _trace=False):
    from concourse.bass_utils import run_bass_kernel_spmd

    x = np.asarray(x, dtype=np.float32)
    W_qkv = np.asarray(W_qkv, dtype=np.float32)
    W_proj = np.asarray(W_proj, dtype=np.float32)

    nc = _get_nc()
    in_maps = [_prep_core_inputs(x, W_qkv, W_proj, cid // 2, cid % 2)
               for cid in range(NCORES)]
    res = run_bass_kernel_spmd(nc, in_maps, core_ids=list(range(NCORES)),
                               trace=_trace)
    _CACHE["last_results"] = res
    out = np.empty((B, T, C), dtype=np.float32)
    for b in range(B):
        out[b] = res.results[2 * b]["out"] + res.results[2 * b + 1]["out"]
    return out


# revision 5
# speedup vs baseline: 1.1013x; 1.0263x over previous
# Causal self-attention (B=4, T=2048, C=1024, H=16, Dh=64) on 8 NeuronCores.
#
# Sharding: core (b, g) = batch b, head-group g (8 heads). Tensor-parallel over
# heads + data-parallel over batch. Each core computes a full [T, C] partial
# projection output; host sums the two head-group partials per batch.
#
# Per-core device program (all matmul operands bf16, fp32 PSUM accumulate):
#   1. QK proj (transposed): QK_T[o, t] for o in [Q(512) | K(512)], streamed
#      over 4 query-chunks of 512 tokens.
#   2. V proj (untransposed): V[t, h*65 + d] with a ones column per head at
#      d=64 (gives softmax denominator for free in the PV matmul).
#   3. Attention per head-pair: S_T[k, q] = K_T.T @ Q_T blocks of
#      [128 keys x 512 queries]; 2 heads packed in the PE array via
#      base-partition 0/64 (row tiling). exp on ScalarE (PSUM->SBUF, bf16),
#      tri-mask applied by DVE multiply on the 128-col diagonal window only.
#   4. PV flipped: y[q, d] (+ denom col 64) = P_T.T @ V_aug per q-subtile of
#      128 (full 128 output partitions, causal subtile skipping), accumulated
#      over key tiles into bank-padded PSUM slots. One PSUM accumulation
#      group per 2KB bank: start on the bank's first write, stop on its last.
#   5. Normalize with per-partition scalar 1/denom on DVE (bf16 z[q, c-pair]),
#      then DMA-XBAR transpose each [128, 128] block back to z_T[c, q].
#   6. Proj: out[t, :] = sum_c z_T[c, t] * Wp[c, :], fp32 out, DMA to HBM.

import numpy as np
import ml_dtypes

B, T, C = 4, 2048, 1024
H, DH = 16, 64
HL = 8            # heads per core
DL = HL * DH      # 512 local head dims
NCORES = 8
TCH = 512         # token chunk (query chunk)
NQC = T // TCH    # 4
NKT = T // 128    # 16 key tiles

BF16 = ml_dtypes.bfloat16

_CACHE = {}


def _build_nc():
    import concourse.bass as bass
    import concourse.tile as tile
    import concourse.mybir as mybir
    from concourse import bacc

    f32 = mybir.dt.float32
    bf16 = mybir.dt.bfloat16
    EXP = mybir.ActivationFunctionType.Exp

    nc = bacc.Bacc("TRN2", target_bir_lowering=False, debug=False)

    # ---- I/O ----
    xt_d = nc.dram_tensor("xt", [C, T], bf16, kind="ExternalInput")
    wq_d = nc.dram_tensor("wq", [C, 1024], bf16, kind="ExternalInput")  # [c, Q|K]
    wv_d = nc.dram_tensor("wv", [C, DL], bf16, kind="ExternalInput")
    wp_d = nc.dram_tensor("wp", [DL, C], bf16, kind="ExternalInput")
    out_d = nc.dram_tensor("out", [T, C], f32, kind="ExternalOutput")

    # lower-triangular block mask: tri[i, j] = 1 if j >= i (applied on the
    # single 128-col diagonal window of each diagonal S block)
    mi = np.arange(128)[:, None]
    mj = np.arange(128)[None, :]
    tri_np = (mj >= mi).astype(BF16)
    tri_d = nc.inline_tensor(np.ascontiguousarray(tri_np), name="tri")

    with tile.TileContext(nc) as tc:
        with (
            tc.tile_pool(name="persist", bufs=1) as persist,
            tc.tile_pool(name="xtp", bufs=4) as xtp,
            tc.tile_pool(name="pp", bufs=8) as pp,
            tc.tile_pool(name="zqp", bufs=6) as zqp,
            tc.tile_pool(name="recp", bufs=3) as recp,
            tc.tile_pool(name="outp", bufs=4) as outp,
            tc.tile_pool(name="sps", bufs=2, space="PSUM") as spsp,
            tc.tile_pool(name="fps", bufs=2, space="PSUM") as fpsp,
            tc.tile_pool(name="pvps", bufs=1, space="PSUM") as pvpsp,
        ):
            # ---- persistent tiles (wq first: first QK matmul depends on it) ----
            wq_sb = persist.tile([128, 8, 1024], bf16, tag="wq", name="wq")
            wv_sb = persist.tile([128, 8, DL], bf16, tag="wv", name="wv")
            wp_sb = persist.tile([128, 4, 1024], bf16, tag="wp", name="wp")
            tri_sb = persist.tile([128, 128], bf16, tag="tri", name="tri")
            # few big 3D-AP DMAs; cc0 of wq separate so the first matmul
            # unblocks as early as possible
            wq_r = wq_d.rearrange("(cc p) o -> p cc o", p=128)
            wv_r = wv_d.rearrange("(cc p) o -> p cc o", p=128)
            wp_r = wp_d.rearrange("(cc p) o -> p cc o", p=128)

            # QK_T pair tiles: [128, 2*TCH]; halves = o-tiles 2jp, 2jp+1
            # (jp<2: Q head-pairs; jp>=2: K head-pairs)
            qk_sb = [[persist.tile([128, 2 * TCH], bf16, tag=f"qk_{j}_{q}", name=f"qk_{j}_{q}")
                      for q in range(NQC)] for j in range(4)]
            # V pair tiles: [128 keys, 2 key-tiles, 8 heads, 65]
            v_sb = [persist.tile([128, 2, HL, DH + 1], bf16, tag=f"v_{t}", name=f"v_{t}")
                    for t in range(NKT // 2)]
            # z_T = normalized y_T: per (head-pair, qc): rows 0-63 head 2p,
            # 64-127 head 2p+1, [128 c, TCH q]
            z_sb = [[persist.tile([128, TCH], bf16, tag=f"z_{p}_{q}", name=f"z_{p}_{q}")
                     for q in range(NQC)] for p in range(4)]

            # input DMAs ordered by first use: the transfer engine is a
            # serial resource, so earliest-needed bytes go first
            xt_r = xt_d.rearrange("(cc p) t -> p cc t", p=128)
            xt_tiles = [xtp.tile([128, 8, TCH], bf16, tag="xt", name="xt")
                        for _ in range(NQC)]
            nc.sync.dma_start(out=wq_sb[:, 0, :], in_=wq_r[:, 0, :])
            nc.sync.dma_start(out=xt_tiles[0][:, 0, :], in_=xt_r[:, 0, 0:TCH])
            for cc in range(1, 8):
                nc.sync.dma_start(out=wq_sb[:, cc, :], in_=wq_r[:, cc, :])
                nc.sync.dma_start(out=xt_tiles[0][:, cc, :], in_=xt_r[:, cc, 0:TCH])
            nc.sync.dma_start(out=wv_sb, in_=wv_r)
            nc.sync.dma_start(out=tri_sb, in_=tri_d[:])
            nc.sync.dma_start(out=wp_sb, in_=wp_r)
            for qc in range(1, NQC):
                nc.sync.dma_start(out=xt_tiles[qc],
                                  in_=xt_r[:, :, qc * TCH:(qc + 1) * TCH])

            # Filler units are half-sized (one 512-wide PSUM bank, 8 or 4
            # matmuls) so they interleave finely against the exp-latency
            # deficit of the S chain.
            def emit_proj_half(qc, u, half):
                tt = qc * 4 + u
                ps_t = fpsp.tile([128, 512], f32, tag="f", name="ps")
                for cp in range(4):
                    nc.tensor.matmul(
                        ps_t,
                        lhsT=z_sb[cp][qc][:, u * 128:(u + 1) * 128],
                        rhs=wp_sb[:, cp, half * 512:(half + 1) * 512],
                        start=(cp == 0), stop=(cp == 3))
                o_t = outp.tile([128, 512], f32, tag="o", name="o")
                nc.vector.tensor_copy(out=o_t, in_=ps_t)
                nc.sync.dma_start(
                    out=out_d[tt * 128:(tt + 1) * 128, half * 512:(half + 1) * 512],
                    in_=o_t)

            def emit_proj(qc):
                for u in range(4):
                    for half in range(2):
                        emit_proj_half(qc, u, half)

            def emit_qk_half(qc, jp, half):
                xt_t = xt_tiles[qc]
                j = 2 * jp + half
                ps_t = fpsp.tile([128, 512], f32, tag="f", name="ps")
                for cc in range(8):
                    nc.tensor.matmul(
                        ps_t,
                        lhsT=wq_sb[:, cc, j * 128:(j + 1) * 128],
                        rhs=xt_t[:, cc, :],
                        start=(cc == 0), stop=(cc == 7))
                nc.vector.tensor_copy(
                    out=qk_sb[jp][qc][:, half * 512:(half + 1) * 512], in_=ps_t)

            def emit_v_half(qc, up, half):
                xt_t = xt_tiles[qc]
                ps_t = fpsp.tile([128, 512], f32, tag="f", name="ps")
                for cc in range(8):
                    nc.tensor.matmul(
                        ps_t,
                        lhsT=xt_t[:, cc, (2 * up + half) * 128:(2 * up + half + 1) * 128],
                        rhs=wv_sb[:, cc, :],
                        start=(cc == 0), stop=(cc == 7))
                tp = qc * 2 + up
                nc.vector.tensor_copy(out=v_sb[tp][:, half, :, 0:DH], in_=ps_t)
                nc.gpsimd.memset(v_sb[tp][:, half, :, DH:DH + 1], 1.0)

            # QK(0) eagerly, ordered so the first S matmul (Q jp0 / K jp2)
            # unblocks earliest; V(0) interleaves into the first attention
            # iterations (PV(kt) only needs V half-units just-in-time)
            for jp in (0, 2, 1, 3):
                for half in range(2):
                    emit_qk_half(0, jp, half)

            for qc in range(NQC):
                # ---------- attention for query chunk qc ----------
                # Fillers interleaved between head-pairs keep PE busy while
                # attention waits on the exp pipeline. proj(0..2) all land in
                # window 3 (the only ACT-bound window with PE slack).
                fillers = []
                if qc == 0:
                    fillers += [lambda up=up, h=h: emit_v_half(0, up, h)
                                for up in range(2) for h in range(2)]
                if qc == 3:
                    fillers += [lambda pq=pq, u=u, h=h: emit_proj_half(pq, u, h)
                                for pq in (0, 1, 2) for u in range(4) for h in range(2)]
                if qc + 1 < NQC:
                    fillers += [lambda jp=jp, h=h: emit_qk_half(qc + 1, jp, h)
                                for jp in range(4) for h in range(2)]
                    fillers += [lambda up=up, h=h: emit_v_half(qc + 1, up, h)
                                for up in range(2) for h in range(2)]

                nkt = 4 * qc + 4  # causal: key tiles 0..4qc+3
                total_iters = 4 * nkt
                it = emitted = 0
                for hp in range(4):
                    # PV accumulators: [128 q, hh, s, 65 of 128] f32; the s
                    # slots of one hh share a 2KB bank = one accumulation
                    # group (start on first write, stop on last)
                    y_ps = pvpsp.tile([128, 2, 4, 128], f32, tag="pv", name="pv")
                    for kt in range(nkt):
                        # s_t halves = the two heads of the pair at the SAME
                        # key tile, so both share causal offset m and the exp
                        # can be narrowed with one 3D AP on diagonal blocks
                        m = kt - 4 * qc
                        w0 = 128 * m if m > 0 else 0
                        s_t = spsp.tile([128, 1024], f32, tag="s", name="s")
                        for hh in range(2):
                            nc.tensor.matmul(
                                s_t[:, hh * 512 + w0:(hh + 1) * 512],
                                lhsT=qk_sb[2 + hp // 2][kt // 4][
                                    hh * 64:(hh + 1) * 64,
                                    (hp % 2) * 512 + (kt % 4) * 128:
                                    (hp % 2) * 512 + ((kt % 4) + 1) * 128],
                                rhs=qk_sb[hp // 2][qc][hh * 64:(hh + 1) * 64,
                                                       (hp % 2) * 512 + w0:(hp % 2 + 1) * 512],
                                start=True, stop=True)
                        p_t = pp.tile([128, 1024], bf16, tag="p", name="p")
                        if w0:
                            nc.scalar.activation(
                                out=p_t.rearrange("p (h w) -> p h w", h=2)[:, :, w0:],
                                in_=s_t.rearrange("p (h w) -> p h w", h=2)[:, :, w0:],
                                func=EXP)
                        else:
                            nc.scalar.activation(out=p_t, in_=s_t, func=EXP)
                        if m >= 0:  # diagonal block: mask the 128-col window
                            for hh in range(2):
                                nc.vector.tensor_mul(
                                    p_t[:, hh * 512 + w0:hh * 512 + w0 + 128],
                                    p_t[:, hh * 512 + w0:hh * 512 + w0 + 128],
                                    tri_sb)
                        # spread filler work (prev proj + next QKV) evenly
                        # through the attention window, emitted BETWEEN the S
                        # and PV matmuls so it can hide the exp latency in the
                        # FIFO engine stream
                        it += 1
                        while emitted < len(fillers) * it // total_iters:
                            fillers[emitted]()
                            emitted += 1
                        # flipped PV: per q-subtile s of 128, skip subtiles
                        # that precede this key tile (causal)
                        for hh in range(2):
                            h = 2 * hp + hh
                            for s in range(4):
                                if 4 * qc + s < kt:
                                    continue
                                nc.tensor.matmul(
                                    y_ps[:, hh, s, 0:DH + 1],
                                    lhsT=p_t[:, hh * 512 + s * 128:hh * 512 + (s + 1) * 128],
                                    rhs=v_sb[kt // 2][:, kt % 2, h, :],
                                    start=(kt == 0 and s == 0),
                                    stop=(kt == nkt - 1 and s == 3))
                    # normalize: z[q, hh*64+d] = y[q, hh, s, d] / y[q, hh, s, 64]
                    rec_t = recp.tile([128, 2, 4], f32, tag="rec", name="rec")
                    for hh in range(2):
                        nc.vector.reciprocal(out=rec_t[:, hh, :],
                                             in_=y_ps[:, hh, :, DH])
                    for s in range(4):
                        zq_t = zqp.tile([128, 2, DH], bf16, tag="zq", name="zq")
                        nc.vector.tensor_mul(
                            zq_t,
                            y_ps[:, :, s, 0:DH],
                            rec_t[:, :, s:s + 1].broadcast_to([128, 2, DH]))
                        # z_T[c, q] block via DMA-XBAR transpose (no PE/PSUM)
                        nc.sync.dma_start_transpose(
                            out=z_sb[hp][qc][:, s * 128:(s + 1) * 128],
                            in_=zq_t.rearrange("p a b -> p (a b)"))
                while emitted < len(fillers):
                    fillers[emitted]()
                    emitted += 1

            emit_proj(NQC - 1)

    nc.compile()
    return nc


def _get_nc():
    if "nc" not in _CACHE:
        _CACHE["nc"] = _build_nc()
    return _CACHE["nc"]


def _prep_core_inputs(x, W_qkv, W_proj, b, g):
    xt = np.ascontiguousarray(x[b].T).astype(BF16)                    # [C, T]
    wq_rows = W_qkv[g * DL:(g + 1) * DL, :] * (1.0 / np.sqrt(DH))     # Q, pre-scaled
    wk_rows = W_qkv[C + g * DL:C + (g + 1) * DL, :]
    wq = np.ascontiguousarray(np.concatenate([wq_rows, wk_rows], 0).T).astype(BF16)
    wv = np.ascontiguousarray(W_qkv[2 * C + g * DL:2 * C + (g + 1) * DL, :].T).astype(BF16)
    wp = np.ascontiguousarray(W_proj[:, g * DL:(g + 1) * DL].T).astype(BF16)
    return {"xt": xt, "wq": wq, "wv": wv, "wp": wp}


def kernel(x, W_qkv, W_proj, _trace=False):
    from concourse.bass_utils import run_bass_kernel_spmd

    x = np.asarray(x, dtype=np.float32)
    W_qkv = np.asarray(W_qkv, dtype=np.float32)
    W_proj = np.asarray(W_proj, dtype=np.float32)

    nc = _get_nc()
    in_maps = [_prep_core_inputs(x, W_qkv, W_proj, cid // 2, cid % 2)
               for cid in range(NCORES)]
    res = run_bass_kernel_spmd(nc, in_maps, core_ids=list(range(NCORES)),
                               trace=_trace)
    _CACHE["last_results"] = res
    out = np.empty((B, T, C), dtype=np.float32)
    for b in range(B):
        out[b] = res.results[2 * b]["out"] + res.results[2 * b + 1]["out"]
    return out
